# revision 17
# baseline (speedup 1.0000x reference)
"""Trainium2 Bass kernel for DirectInterpGNN message passing.

Math (per reference):
    num_v  = sum_{e: src_e=v} A_e
    den_v  = sum_{e: src_e=v} A_e*S_e*v_e
    f_v    = (C_v - 1) * (num_v/den_v) / A_ii_v
    w_e    = A_e * f_{src_e}

Sharding strategy (chosen; the hint's edge-split + all-reduce is one option,
this uses vertex-range sharding instead): edges are sorted by source vertex on
the host and split across the 8 cores at vertex boundaries, so each core owns
a disjoint contiguous vertex range and ALL edges incident to it. No collective
is needed.

Within a core's shard the host lays edges out in exact-degree classes: for
each per-core degree d, the n_d vertices of that degree occupy m_d = ceil(
n_d/128) slots per partition, their d edges contiguous in the free dimension.
The per-vertex segment sums then become plain innermost-axis tensor_reduce
calls, f is computed densely per vertex slot, and w_e = A_e * f is a single
broadcast multiply (0-stride AP along the degree axis). The device program is
fully dense: no indirect DMA, no dedup, no cross-partition traffic.

The device program's structure depends only on the class grid (list of
(d, m_d)) which is derived from the input degree histogram; compiled kernels
are cached by that grid.
"""
import sys
sys.path.insert(0, '/opt/trn_rl_repo')
sys.path.insert(0, '/root/.axon_site/_ro/trn_rl_repo')

import numpy as np
import ml_dtypes

P = 128
USE_BF16 = True
N_CORES = 8


# ---------------------------------------------------------------- host layout

def _build_layout(src, n_vertices):
    """Compute the sharding + degree-class layout for the given edge list.

    Returns a dict with the global class grid and per-core placement arrays.
    """
    E = src.shape[0]
    order = np.argsort(src, kind="stable")
    ssorted = src[order]

    # shard boundaries at vertex-run starts nearest to equal eighths
    bounds = [0]
    for c in range(1, N_CORES):
        t = (E * c) // N_CORES
        v = ssorted[t]
        b = int(np.searchsorted(ssorted, v, side="left"))
        bounds.append(b)
    bounds.append(E)

    cores = []
    for c in range(N_CORES):
        lo, hi = bounds[c], bounds[c + 1]
        seg_src = ssorted[lo:hi]
        vs, first, counts = np.unique(
            seg_src, return_index=True, return_counts=True)
        cores.append(dict(lo=lo, hi=hi, vs=vs, first=first, counts=counts))

    # global class grid: union of degrees, m_d = max over cores
    all_d = sorted({int(d) for core in cores for d in np.unique(core["counts"])})
    m_of = {}
    for d in all_d:
        m = 0
        for core in cores:
            n_d = int((core["counts"] == d).sum())
            m = max(m, -(-n_d // P))
        m_of[d] = m
    classes = [(d, m_of[d]) for d in all_d]
    FE = sum(d * m for d, m in classes)
    FV = sum(m for _, m in classes)
    eoff, voff = {}, {}
    e, v = 0, 0
    for d, m in classes:
        eoff[d] = e
        voff[d] = v
        e += d * m
        v += m

    # per-core placement
    for core in cores:
        vs, first, counts = core["vs"], core["first"], core["counts"]
        nv = len(vs)
        # order vertices by (degree, id); rank within class
        ordv = np.lexsort((vs, counts))
        rank = np.empty(nv, np.int64)
        d_ord = counts[ordv]
        grp_start = np.r_[0, np.flatnonzero(np.diff(d_ord)) + 1]
        within = np.arange(nv) - np.repeat(
            grp_start, np.diff(np.r_[grp_start, nv]))
        rank[ordv] = within
        p_v = rank % P
        i_v = rank // P
        dd = counts.astype(np.int64)
        off_e_v = np.array([eoff[int(d)] for d in dd], np.int64)
        off_v_v = np.array([voff[int(d)] for d in dd], np.int64)
        vcol = off_v_v + i_v
        ebase = off_e_v + i_v * dd
        # expand per edge
        j = np.arange(core["hi"] - core["lo"],
                      dtype=np.int64) - np.repeat(first, counts)
        part_e = np.repeat(p_v, counts)
        col_e = np.repeat(ebase, counts) + j
        core["flat_e"] = part_e * FE + col_e
        core["flat_v"] = p_v * FV + vcol
        core["edge_ids"] = order[core["lo"]:core["hi"]]

    key = tuple(classes)
    return dict(classes=classes, FE=FE, FV=FV, cores=cores, key=key, E=E)


def _make_in_maps(vertex_attr, edge_attr, layout, use_bf16=USE_BF16):
    edt = ml_dtypes.bfloat16 if use_bf16 else np.float32
    FE, FV = layout["FE"], layout["FV"]
    in_maps = []
    for core in layout["cores"]:
        ids = core["edge_ids"]
        fe = core["flat_e"]
        A = np.zeros(P * FE, edt)
        S = np.ones(P * FE, edt)
        V = np.ones(P * FE, edt)
        A[fe] = edge_attr[ids, 0].astype(edt)
        S[fe] = edge_attr[ids, 1].astype(edt)
        V[fe] = edge_attr[ids, 2].astype(edt)
        AII = np.ones(P * FV, np.float32)
        CC = np.ones(P * FV, np.float32)
        fv = core["flat_v"]
        AII[fv] = vertex_attr[core["vs"], 0]
        CC[fv] = vertex_attr[core["vs"], 1]
        in_maps.append({
            "ea": A.reshape(P, FE),
            "es": S.reshape(P, FE),
            "ev": V.reshape(P, FE),
            "vai": AII.reshape(P, FV),
            "vc": CC.reshape(P, FV),
        })
    return in_maps


# ------------------------------------------------------------- device program

def _make_pieces(classes, cw_max):
    """Split classes into (d, m_piece) pieces of width <= cw_max, then pack
    consecutive pieces into super-chunks of total width <= cw_max."""
    pieces = []
    for d, m in classes:
        assert d <= cw_max
        mrem = m
        while mrem:
            mw = min(mrem, max(1, cw_max // d))
            pieces.append((d, mw))
            mrem -= mw
    chunks = []
    cur, curw = [], 0
    for d, mw in pieces:
        w = d * mw
        if cur and curw + w > cw_max:
            chunks.append(cur)
            cur, curw = [], 0
        cur.append((d, mw))
        curw += w
    if cur:
        chunks.append(cur)
    return chunks


def build_kernel(classes, FE, FV, n_cores=N_CORES, repeats=1,
                 cw_max=3072, bufs=4, pool_mult=True, dma_split=True,
                 use_bf16=True, den_tree=False, fold=False):
    from contextlib import nullcontext

    import concourse.bacc as bacc
    import concourse.mybir as mybir
    import concourse.tile as tile

    f32 = mybir.dt.float32
    bf16 = mybir.dt.bfloat16
    edt = bf16 if use_bf16 else f32
    mult = mybir.AluOpType.mult
    nc = bacc.Bacc("TRN2", target_bir_lowering=False, debug=False,
                   num_devices=n_cores)
    ea = nc.dram_tensor("ea", [P, FE], edt, kind="ExternalInput")
    es = nc.dram_tensor("es", [P, FE], edt, kind="ExternalInput")
    ev = nc.dram_tensor("ev", [P, FE], edt, kind="ExternalInput")
    vai = nc.dram_tensor("vai", [P, FV], f32, kind="ExternalInput")
    vc = nc.dram_tensor("vc", [P, FV], f32, kind="ExternalInput")
    w = nc.dram_tensor("w", [P, FE], edt, kind="ExternalOutput")

    chunks = _make_pieces(classes, cw_max)
    eng_es = nc.scalar if dma_split else nc.sync
    eng_ev = nc.scalar if dma_split else nc.sync
    eng_m = nc.gpsimd if pool_mult else nc.vector

    with tile.TileContext(nc) as tc:
        with (tc.tile_pool(name="const", bufs=1) as cpool,
              tc.tile_pool(name="stream", bufs=bufs) as spool,
              tc.tile_pool(name="small", bufs=bufs) as vpool,
              tc.For_i(0, repeats, 1) if repeats > 1 else nullcontext()):
            aii_t = cpool.tile([P, FV], f32)
            nc.sync.dma_start(aii_t[:], vai[:])
            # cm1 = C - 1, computed once for the whole vertex table
            cc_t = cpool.tile([P, FV], f32)
            nc.scalar.dma_start(cc_t[:], vc[:])
            cm1_t = cpool.tile([P, FV], f32)
            nc.vector.tensor_scalar(
                out=cm1_t[:], in0=cc_t[:], scalar1=-1.0, scalar2=None,
                op0=mybir.AluOpType.add)

            eo = 0
            vo = 0
            for chunk_i, chunk in enumerate(chunks):
                cw = sum(d * mw for d, mw in chunk)
                mw_tot = sum(mw for _, mw in chunk)
                a_t = spool.tile([P, cw], edt, tag="a")
                nc.sync.dma_start(a_t[:], ea[:, eo:eo + cw])
                s_t = spool.tile([P, cw], edt, tag="s")
                eng_es.dma_start(s_t[:], es[:, eo:eo + cw])
                v_t = spool.tile([P, cw], edt, tag="v")
                eng_ev.dma_start(v_t[:], ev[:, eo:eo + cw])
                # m = a*s*v, in place over s_t then v_t
                eng_m.tensor_tensor(out=s_t[:], in0=a_t[:], in1=s_t[:], op=mult)
                eng_m.tensor_tensor(out=v_t[:], in0=s_t[:], in1=v_t[:], op=mult)

                num_t = vpool.tile([P, mw_tot], f32, tag="num")
                den_t = vpool.tile([P, mw_tot], f32, tag="den")

                def fold_reduce(src3, dst, d, mw, tag):
                    # level-1 pairwise fold on Pool (bf16+bf16 -> fp32, exact)
                    # then DVE reduce over the halved width
                    add = mybir.AluOpType.add
                    h = d // 2
                    r = d - h
                    if d == 2:
                        eng_m.tensor_tensor(
                            out=dst,
                            in0=src3[:, :, 0:1].rearrange("p m o -> p (m o)"),
                            in1=src3[:, :, 1:2].rearrange("p m o -> p (m o)"),
                            op=add)
                        return
                    hv = spool.tile([P, mw * r], f32, tag=tag)
                    hv3 = hv[:].rearrange("p (m r) -> p m r", r=r)
                    if d % 2 == 0:
                        eng_m.tensor_tensor(
                            out=hv3, in0=src3[:, :, 0:h], in1=src3[:, :, h:d],
                            op=add)
                    else:
                        eng_m.tensor_tensor(
                            out=hv3[:, :, 1:r], in0=src3[:, :, 1:1 + h],
                            in1=src3[:, :, 1 + h:d], op=add)
                        eng_m.tensor_copy(hv3[:, :, 0:1], src3[:, :, 0:1])
                    nc.vector.tensor_reduce(
                        out=dst, in_=hv3, axis=mybir.AxisListType.X,
                        op=add)

                co = 0
                po = 0
                for d, mw in chunk:
                    if d > 1 and fold:
                        fold_reduce(
                            a_t[:, co:co + mw * d].rearrange(
                                "p (m d) -> p m d", d=d),
                            num_t[:, po:po + mw], d, mw, "hn")
                        fold_reduce(
                            v_t[:, co:co + mw * d].rearrange(
                                "p (m d) -> p m d", d=d),
                            den_t[:, po:po + mw], d, mw, "hd")
                        co += mw * d
                        po += mw
                        continue
                    if d > 1:
                        nc.vector.tensor_reduce(
                            out=num_t[:, po:po + mw],
                            in_=a_t[:, co:co + mw * d].rearrange(
                                "p (m d) -> p m d", d=d),
                            axis=mybir.AxisListType.X,
                            op=mybir.AluOpType.add)
                        if den_tree:
                            # pairwise in-place add-tree over the m tile on
                            # Pool; column 0 of each d-block ends up with den
                            bv = v_t[:, co:co + mw * d].rearrange(
                                "p (m d) -> p m d", d=d)
                            width = d
                            while width > 1:
                                h = width // 2
                                eng_m.tensor_tensor(
                                    out=bv[:, :, 0:h], in0=bv[:, :, 0:h],
                                    in1=bv[:, :, width - h:width],
                                    op=mybir.AluOpType.add)
                                width -= h
                            nc.vector.tensor_copy(
                                den_t[:, po:po + mw], bv[:, :, 0:1].rearrange(
                                    "p m o -> p (m o)"))
                        else:
                            nc.vector.tensor_reduce(
                                out=den_t[:, po:po + mw],
                                in_=v_t[:, co:co + mw * d].rearrange(
                                    "p (m d) -> p m d", d=d),
                                axis=mybir.AxisListType.X,
                                op=mybir.AluOpType.add)
                    else:
                        nc.vector.tensor_copy(
                            num_t[:, po:po + mw], a_t[:, co:co + mw])
                        nc.vector.tensor_copy(
                            den_t[:, po:po + mw], v_t[:, co:co + mw])
                    co += mw * d
                    po += mw

                # f = cm1 * num / (den_safe * A_ii) for the whole chunk
                dsafe = vpool.tile([P, mw_tot], f32, tag="dsafe")
                nc.vector.tensor_scalar(
                    out=dsafe[:], in0=den_t[:], scalar1=0.0, scalar2=None,
                    op0=mybir.AluOpType.is_equal)
                nc.vector.tensor_tensor(
                    out=dsafe[:], in0=dsafe[:], in1=den_t[:],
                    op=mybir.AluOpType.add)
                nc.vector.tensor_tensor(
                    out=dsafe[:], in0=dsafe[:], in1=aii_t[:, vo:vo + mw_tot],
                    op=mult)
                nc.vector.reciprocal(out=dsafe[:], in_=dsafe[:])
                nc.vector.tensor_tensor(
                    out=num_t[:], in0=num_t[:], in1=dsafe[:], op=mult)
                nc.vector.tensor_tensor(
                    out=num_t[:], in0=num_t[:], in1=cm1_t[:, vo:vo + mw_tot],
                    op=mult)

                # w = a * f (broadcast f along the degree axis), reuse s_t
                if use_bf16:
                    fb_t = vpool.tile([P, mw_tot], edt, tag="fb")
                    nc.vector.tensor_copy(fb_t[:], num_t[:])
                    f_src = fb_t
                else:
                    f_src = num_t
                co = 0
                po = 0
                for d, mw in chunk:
                    if d > 1:
                        f_b = f_src[:, po:po + mw].rearrange(
                            "p (m o) -> p m o", o=1).to_broadcast([P, mw, d])
                        (nc.vector if fold else eng_m).tensor_tensor(
                            out=s_t[:, co:co + mw * d].rearrange(
                                "p (m d) -> p m d", d=d),
                            in0=a_t[:, co:co + mw * d].rearrange(
                                "p (m d) -> p m d", d=d),
                            in1=f_b, op=mult)
                    else:
                        (nc.vector if fold else eng_m).tensor_tensor(
                            out=s_t[:, co:co + mw], in0=a_t[:, co:co + mw],
                            in1=f_src[:, po:po + mw], op=mult)
                    co += mw * d
                    po += mw
                eng_w = nc.sync if (chunk_i % 2 == 0) else nc.scalar
                eng_w.dma_start(w[:, eo:eo + cw], s_t[:])

                eo += cw
                vo += mw_tot
            assert eo == FE and vo == FV

    nc.compile()
    return nc


# ------------------------------------------------------------------- wrapper

_CACHE = {}


def _get_kernel(layout):
    key = (layout["key"], USE_BF16)
    if key not in _CACHE:
        _CACHE[key] = build_kernel(layout["classes"], layout["FE"],
                                   layout["FV"], use_bf16=USE_BF16)
    return _CACHE[key]


def kernel(vertex_attr, edge_attr, edgeij_pair):
    from concourse.bass_utils import run_bass_kernel_spmd

    vertex_attr = np.asarray(vertex_attr, dtype=np.float32)
    edge_attr = np.asarray(edge_attr, dtype=np.float32)
    src = np.ascontiguousarray(np.asarray(edgeij_pair, dtype=np.int32)[0])

    layout = _build_layout(src, vertex_attr.shape[0])
    nc = _get_kernel(layout)
    in_maps = _make_in_maps(vertex_attr, edge_attr, layout)
    res = run_bass_kernel_spmd(nc, in_maps, list(range(N_CORES)))

    out = np.empty(layout["E"], np.float32)
    for c, core in enumerate(layout["cores"]):
        wp = np.asarray(res.results[c]["w"]).astype(np.float32).reshape(-1)
        out[core["edge_ids"]] = wp[core["flat_e"]]
    return out
